# revision 1
# baseline (speedup 1.0000x reference)
"""Trainium2 Bass kernel: Mixtral-style per-expert SwiGLU MLP.

Reference computation (E=8 experts, B=2, C=1024, M=2048, H=7168):
    gate = einsum("ebcm,emh->ebch", dispatch_input, w1)
    up   = einsum("ebcm,emh->ebch", dispatch_input, w3)
    out  = einsum("ebch,ehm->ebcm", silu(gate) * up, w2)

Sharding: expert-parallel across the 8 NeuronCores — core e handles expert e's
full MLP (T = B*C = 2048 tokens, no collectives needed).

Per-core kernel (all matmuls bf16, fp32 accumulation in PSUM):
  - X [T, M] is transposed on the TensorEngine into XT [M, T] (bf16) so the
    contraction dim m lands on SBUF partitions.
  - gate^T/up^T [h, t] tiles: stationary = w1/w3 column blocks [m128, h128]
    (cast to bf16 in-flight by SWDGE DMA), moving = XT [m128, t512].
  - hidden^T = silu(gate^T) * up^T stored bf16 in SBUF, [h, t] layout.
  - down proj: stationary = w2 blocks [h128, m128], moving = hidden^T
    [h128, t512]; accumulated over h. Output is produced in [M, T] layout
    (out^T); the host transposes for free during the gather.
  - t is processed in 2 blocks of 1024 and h in 2 halves of 3584 so hidden^T
    and the partial-output accumulator fit in SBUF. The next t-block's X
    transposes are emitted right after the last gate/up read of the current
    XT so the PE never stalls on X DMAs at the block boundary.
"""

import numpy as np

import concourse.bass as bass
import concourse.mybir as mybir
import concourse.tile as tile
from concourse import bacc
from concourse.bass_utils import run_bass_kernel_spmd
from concourse.masks import make_identity

E = 8
B, C = 2, 1024
T = B * C          # 2048 tokens per expert
M = 2048           # model dim (contraction for gate/up)
H = 7168           # ffn dim (contraction for down)
P = 128
TB = 1024          # t-block (2 blocks)
N_TB = T // TB
TS = 512           # moving free-dim per matmul (1 PSUM bank fp32)
N_TS = TB // TS
MT = M // P        # 16 m-tiles
HT = H // P        # 56 h-tiles
HHALF = HT // 2    # 28 h-tiles per half
F32 = mybir.dt.float32
BF16 = mybir.dt.bfloat16

_NC_CACHE = {}


def _build_nc():
    nc = bacc.Bacc("TRN2", target_bir_lowering=False)
    x = nc.dram_tensor("x", [T, M], F32, kind="ExternalInput")
    w1 = nc.dram_tensor("w1", [M, H], F32, kind="ExternalInput")
    w3 = nc.dram_tensor("w3", [M, H], F32, kind="ExternalInput")
    w2 = nc.dram_tensor("w2", [H, M], F32, kind="ExternalInput")
    out = nc.dram_tensor("out", [M, T], F32, kind="ExternalOutput")

    with tile.TileContext(nc) as tc:
        with (
            tc.tile_pool(name="consts", bufs=1) as consts,
            tc.tile_pool(name="xtp", bufs=1) as xtp,
            tc.tile_pool(name="hidp", bufs=1) as hidp,
            tc.tile_pool(name="oaccp", bufs=1) as oaccp,
            tc.tile_pool(name="xinp", bufs=6) as xinp,
            tc.tile_pool(name="wp", bufs=4) as wp,
            tc.tile_pool(name="w2p", bufs=3) as w2p,
            tc.tile_pool(name="sgp", bufs=2) as sgp,
            tc.tile_pool(name="outp", bufs=2) as outp,
            tc.tile_pool(name="psp", bufs=8, space="PSUM") as psp,
        ):
            ident = consts.tile([P, P], F32)
            make_identity(nc, ident)

            def emit_transpose(tb):
                """X[t-block] -> XT [m, t] bf16, via PE transpose. tt-major
                order so the first transpose only waits on one 128KB DMA."""
                t0 = tb * TB
                xt = xtp.tile([P, MT, TB], BF16, tag="xt", name=f"xt{tb}")
                for tt in range(TB // P):
                    for mp in range(MT // 2):
                        xin = xinp.tile([P, 2 * P], F32, tag="xin", name="xin")
                        nc.sync.dma_start(
                            out=xin,
                            in_=x[t0 + tt * P : t0 + (tt + 1) * P,
                                  mp * 2 * P : (mp + 1) * 2 * P],
                        )
                        for k in range(2):
                            mt = 2 * mp + k
                            pst = psp.tile([P, P], F32, tag="ps", name="pst")
                            nc.tensor.transpose(
                                pst, xin[:, k * P : (k + 1) * P], ident
                            )
                            nc.scalar.copy(
                                out=xt[:, mt, tt * P : (tt + 1) * P], in_=pst
                            )
                return xt

            def emit_gate_up(tb, half, xt):
                """gate/up matmuls + silu*mul -> hidden^T bf16 for one h-half."""
                h0 = half * HHALF
                hid = hidp.tile([P, HHALF, TB], BF16, tag="hid", name="hid")
                for hl in range(HHALF):
                    ht = h0 + hl
                    w1b = wp.tile([P, MT, P], BF16, tag="w1b", name="w1b")
                    nc.gpsimd.dma_start(
                        out=w1b,
                        in_=w1[:, ht * P : (ht + 1) * P].rearrange(
                            "(mo mi) h -> mi mo h", mi=P
                        ),
                    )
                    w3b = wp.tile([P, MT, P], BF16, tag="w3b", name="w3b")
                    nc.gpsimd.dma_start(
                        out=w3b,
                        in_=w3[:, ht * P : (ht + 1) * P].rearrange(
                            "(mo mi) h -> mi mo h", mi=P
                        ),
                    )
                    for ts in range(N_TS):
                        tsl = slice(ts * TS, (ts + 1) * TS)
                        ps_g = psp.tile([P, TS], F32, tag="ps", name="ps_g")
                        for mt in range(MT):
                            nc.tensor.matmul(
                                ps_g,
                                w1b[:, mt],
                                xt[:, mt, tsl],
                                start=(mt == 0),
                                stop=(mt == MT - 1),
                            )
                        ps_u = psp.tile([P, TS], F32, tag="ps", name="ps_u")
                        for mt in range(MT):
                            nc.tensor.matmul(
                                ps_u,
                                w3b[:, mt],
                                xt[:, mt, tsl],
                                start=(mt == 0),
                                stop=(mt == MT - 1),
                            )
                        sg = sgp.tile([P, TS], BF16, tag="sg", name="sg")
                        nc.scalar.activation(
                            sg, ps_g, mybir.ActivationFunctionType.Silu
                        )
                        nc.vector.tensor_mul(hid[:, hl, tsl], sg, ps_u)
                return hid

            def emit_down(tb, half, hid, oacc):
                """down-proj for one h-half; half 0 stages into oacc (bf16),
                half 1 adds and streams out."""
                t0 = tb * TB
                h0 = half * HHALF
                for mt in range(MT):
                    w2b = w2p.tile([P, HHALF, P], BF16, tag="w2b", name="w2b")
                    nc.gpsimd.dma_start(
                        out=w2b,
                        in_=w2[h0 * P : (h0 + HHALF) * P,
                               mt * P : (mt + 1) * P].rearrange(
                            "(ho hi) m -> hi ho m", hi=P
                        ),
                    )
                    for ts in range(N_TS):
                        tsl = slice(ts * TS, (ts + 1) * TS)
                        ps_o = psp.tile([P, TS], F32, tag="ps", name="ps_o")
                        for hl in range(HHALF):
                            nc.tensor.matmul(
                                ps_o,
                                w2b[:, hl],
                                hid[:, hl, tsl],
                                start=(hl == 0),
                                stop=(hl == HHALF - 1),
                            )
                        if half == 0:
                            nc.scalar.copy(out=oacc[:, mt, tsl], in_=ps_o)
                        else:
                            oevac = outp.tile([P, TS], F32, tag="oevac", name="oevac")
                            nc.vector.tensor_add(oevac, ps_o, oacc[:, mt, tsl])
                            nc.sync.dma_start(
                                out=out[mt * P : (mt + 1) * P,
                                        t0 + ts * TS : t0 + (ts + 1) * TS],
                                in_=oevac,
                            )

            xt = emit_transpose(0)
            for tb in range(N_TB):
                oacc = oaccp.tile([P, MT, TB], BF16, tag="oacc", name="oacc")
                hid0 = emit_gate_up(tb, 0, xt)
                emit_down(tb, 0, hid0, oacc)
                hid1 = emit_gate_up(tb, 1, xt)
                # xt's last read is in the gate/up MMs above; emit the next
                # t-block's transposes now so they slot in behind this
                # block's remaining down-proj work with no PE stall.
                if tb + 1 < N_TB:
                    xt_next = emit_transpose(tb + 1)
                emit_down(tb, 1, hid1, oacc)
                if tb + 1 < N_TB:
                    xt = xt_next
    nc.finalize()
    return nc


def _get_nc():
    if "nc" not in _NC_CACHE:
        _NC_CACHE["nc"] = _build_nc()
    return _NC_CACHE["nc"]


def _run(dispatch_input, w1, w2, w3, trace=False):
    nc = _get_nc()
    in_maps = []
    for e in range(E):
        in_maps.append(
            {
                "x": np.ascontiguousarray(
                    np.asarray(dispatch_input[e], dtype=np.float32).reshape(T, M)
                ),
                "w1": np.ascontiguousarray(np.asarray(w1[e], dtype=np.float32)),
                "w3": np.ascontiguousarray(np.asarray(w3[e], dtype=np.float32)),
                "w2": np.ascontiguousarray(np.asarray(w2[e], dtype=np.float32)),
            }
        )
    res = run_bass_kernel_spmd(
        nc, in_maps, core_ids=list(range(E)), trace=trace
    )
    outs = np.stack(
        [np.asarray(r["out"]).T.reshape(B, C, M) for r in res.results]
    )
    return outs.astype(np.float32), res


def kernel(dispatch_input, w1, w2, w3):
    out, _ = _run(dispatch_input, w1, w2, w3, trace=False)
    return out


def kernel_with_trace(dispatch_input, w1, w2, w3):
    return _run(dispatch_input, w1, w2, w3, trace=True)

